# revision 22
# baseline (speedup 1.0000x reference)
"""Multi-head attention (B=2, N=2048, C=768, H=12, DH=64) on 8 Trainium2 cores.

Sharding: data-parallel on batch (cores 0-3 -> b=0, cores 4-7 -> b=1),
tensor-parallel on heads within each group (3 heads/core: Wq/Wk/Wv column
slices, Wp row slices).  Each core emits its partial projection output
[N, C]; the host sums the 4 partials per batch and adds bp (cheaper than a
device collective at this size).

Per-core dataflow (feature-major, transpose-free, fp16 operands / fp32 psum):
  - host supplies xT = x[b].T  [C, N] in fp16; h2's q and k weight columns
    are host-packed into one [C,128] tensor so all qk-proj matmuls are M=128
  - qT,kT [64, N] per head = W.T @ xT; each head's 64 dims then duplicated
    onto both PE-row halves (SBUF->SBUF DMA) so score matmuls pair
    even/odd kj tiles on disjoint PE row halves (co-execute)
  - v [N, 192] token-major from xT as lhsT, with a ones column per head;
    v psum borrowed from the yt pool so the score stream's first groups
    (ST + exp) prefetch underneath the v projection (ET ring, LAG=4)
  - phase 3 is one continuous stream over (qq, head, kj): 192 score tiles
    STt [kj,qi] grouped 3 per [128,1536] psum tile; ONE exp ACT op per
    group; yT accumulation consumes ET four groups behind so transient PE
    detours (proj tiles, normalize) never starve the ACT engine
  - yT_aug[65, qi] = [v_h | 1].T @ ET accumulated over kj; row 64 = denom
  - normalize: denom -> sbuf copy, reciprocal_approx_fast, stride-0 DMA
    broadcast, fused multiply deferred two groups (no DVE head-of-line)
  - out[qi, C] partial = yT (stationary) @ Wp rows, interleaved into the
    next block's stream at slot-parity-preserving spacing; the last four
    row-tiles split h0+h1 (early) / h2 (after the final normalize)
"""

import math

import numpy as np

import concourse.bacc as bacc
import concourse.bass as bass
import concourse.mybir as mybir
import concourse.tile as tile
from concourse import bass_utils

B, N, C, H, DH = 2, 2048, 768, 12, 64
NCORES = 8
CPG = 4                  # cores per batch group
HPC = H // CPG           # heads per core = 3
MYC = HPC * DH           # per-core feature width = 192
KC = C // 128            # contraction chunks = 6
NTT = N // 128           # token tiles = 16
QB = 512                 # qi block (psum bank width, fp32)
LAG = 4                  # ET ring depth: yt consumption trails ACT by LAG
F32 = mybir.dt.float32
MMDT = mybir.dt.float16  # matmul operand dtype
AF = mybir.ActivationFunctionType
OP = mybir.AluOpType

EXP_SHIFT = -4.0         # exp(s + EXP_SHIFT); cancels between num and denom


def _bcast_parts(ap, nparts):
    """Partition-stride-0 broadcast view of a [1, F] AP (DMA source only)."""
    return bass.AP(tensor=ap.tensor, offset=ap.offset,
                   ap=[[0, nparts]] + [list(d) for d in ap.ap[1:]])


def _emit(nc, tc, pools, aps):
    xT, wq, wk, wqk2, wv, wp, bq, bk, bv, out = (
        aps["xT"], aps["wq"], aps["wk"], aps["wqk2"], aps["wv"], aps["wp"],
        aps["bq"], aps["bk"], aps["bv"], aps["out"],
    )
    persist = pools["persist"]
    et_pool = pools["et"]
    small = pools["small"]
    ostage = pools["ostage"]

    # ---- persistent SBUF tensors ----
    xT_sb = persist.tile([128, KC * N], MMDT, tag="xT_sb")
    wq_sb = persist.tile([128, KC * 128], MMDT, tag="wq_sb")
    wk_sb = persist.tile([128, KC * 128], MMDT, tag="wk_sb")
    wqk2_sb = persist.tile([128, KC * 128], MMDT, tag="wqk2_sb")
    wv_sb = persist.tile([128, KC * MYC], MMDT, tag="wv_sb")
    wpA = persist.tile([128, C], MMDT, tag="wpA")
    wpB = persist.tile([64, C], MMDT, tag="wpB")
    bqA = persist.tile([128, 1], F32, tag="bqA")
    bqB = persist.tile([64, 1], F32, tag="bqB")
    bkA = persist.tile([128, 1], F32, tag="bkA")
    bkB = persist.tile([64, 1], F32, tag="bkB")
    bv_row = persist.tile([1, MYC], MMDT, tag="bv_row")
    ones = persist.tile([1, 128], MMDT, tag="ones")
    shift_col = persist.tile([128, 1], F32, tag="shift_col")
    # compact projections (h0 on parts 0:64, h1 on 64:128; h2 separate)
    qTA = persist.tile([128, N], MMDT, tag="qTA")
    kTA = persist.tile([128, N], MMDT, tag="kTA")
    # per-head partition-duplicated k/q for even/odd kj pair packing
    qTD = [persist.tile([128, N], MMDT, tag=f"qTD{h}", name=f"qTD{h}")
           for h in range(HPC)]
    kTD = [persist.tile([128, N], MMDT, tag=f"kTD{h}", name=f"kTD{h}")
           for h in range(HPC)]
    v_sb = persist.tile([128, NTT * HPC * 65], MMDT, tag="v_sb")
    yTA = persist.tile([128, N], MMDT, tag="yTA")
    yTB = persist.tile([64, N], MMDT, tag="yTB")

    # ---- input DMAs: qk-proj operands first so phase 1 starts ASAP ----
    for kc in range(KC):
        nc.sync.dma_start(out=xT_sb[:, kc * N:(kc + 1) * N],
                          in_=xT[kc * 128:(kc + 1) * 128, :])
        nc.sync.dma_start(out=wq_sb[:, kc * 128:(kc + 1) * 128],
                          in_=wq[kc * 128:(kc + 1) * 128, :])
        nc.sync.dma_start(out=wk_sb[:, kc * 128:(kc + 1) * 128],
                          in_=wk[kc * 128:(kc + 1) * 128, :])
        nc.sync.dma_start(out=wqk2_sb[:, kc * 128:(kc + 1) * 128],
                          in_=wqk2[kc * 128:(kc + 1) * 128, :])
    nc.sync.dma_start(out=bqA, in_=bq[0:128, :])
    nc.sync.dma_start(out=bqB, in_=bq[128:MYC, :])
    nc.sync.dma_start(out=bkA, in_=bk[0:128, :])
    nc.sync.dma_start(out=bkB, in_=bk[128:MYC, :])
    for kc in range(KC):
        nc.sync.dma_start(out=wv_sb[:, kc * MYC:(kc + 1) * MYC],
                          in_=wv[kc * 128:(kc + 1) * 128, :])
    nc.sync.dma_start(out=bv_row, in_=bv)
    nc.sync.dma_start(out=wpA, in_=wp[0:128, :])
    nc.sync.dma_start(out=wpB, in_=wp[128:MYC, :])
    ones_f32 = persist.tile([128, 1], F32, tag="ones_f32")
    ones_row_f32 = persist.tile([1, 128], F32, tag="ones_row_f32")
    nc.vector.memset(ones_f32, 1.0)
    nc.vector.memset(ones_row_f32, 1.0)
    nc.vector.tensor_copy(out=ones, in_=ones_row_f32)
    nc.vector.memset(shift_col, EXP_SHIFT)

    # ---- phase 1: q/k/h2-combined projections (M=128 passes) ----
    with tc.tile_pool(name="ps_qk", bufs=2, space="PSUM") as ps_qk:
        pssQ = [ps_qk.tile([128, QB], F32, tag="ps_qkA", bufs=4,
                           name=f"ps_q{_i}") for _i in range(N // QB)]
        pssK = [ps_qk.tile([128, QB], F32, tag="ps_qkB", bufs=4,
                           name=f"ps_k{_i}") for _i in range(N // QB)]
        # q and k interleaved per kc chunk so matmul consumption stays
        # behind the xT DMA supply
        for kc in range(KC):
            for nt in range(N // QB):
                nc.tensor.matmul(
                    pssQ[nt], wq_sb[:, kc * 128:(kc + 1) * 128],
                    xT_sb[:, kc * N + nt * QB: kc * N + nt * QB + QB],
                    start=(kc == 0), stop=(kc == KC - 1))
            for nt in range(N // QB):
                nc.tensor.matmul(
                    pssK[nt], wk_sb[:, kc * 128:(kc + 1) * 128],
                    xT_sb[:, kc * N + nt * QB: kc * N + nt * QB + QB],
                    start=(kc == 0), stop=(kc == KC - 1))
        for nt in range(N // QB):
            nc.vector.tensor_scalar(
                out=qTA[:, nt * QB:(nt + 1) * QB], in0=pssQ[nt],
                scalar1=bqA, scalar2=None, op0=OP.add)
            nc.vector.tensor_scalar(
                out=kTA[:, nt * QB:(nt + 1) * QB], in0=pssK[nt],
                scalar1=bkA, scalar2=None, op0=OP.add)
        # combined h2 pass: psum rows 0:64 = q-h2, rows 64:128 = k-h2
        pss2 = [ps_qk.tile([128, QB], F32, tag="ps_qkA", bufs=4,
                           name=f"ps_2{_i}") for _i in range(N // QB)]
        for kc in range(KC):
            for nt in range(N // QB):
                nc.tensor.matmul(
                    pss2[nt], wqk2_sb[:, kc * 128:(kc + 1) * 128],
                    xT_sb[:, kc * N + nt * QB: kc * N + nt * QB + QB],
                    start=(kc == 0), stop=(kc == KC - 1))
        for nt in range(N // QB):
            nc.vector.tensor_scalar(
                out=qTD[2][0:64, nt * QB:(nt + 1) * QB], in0=pss2[nt][0:64, :],
                scalar1=bqB, scalar2=None, op0=OP.add)
            nc.vector.tensor_scalar(
                out=kTD[2][0:64, nt * QB:(nt + 1) * QB],
                in0=pss2[nt][64:128, :],
                scalar1=bkB, scalar2=None, op0=OP.add)
        # duplicate each head's 64 dims onto both partition halves; DVE
        # and gpsimd copies (4x SBUF fp16 mode) avoid the loaded DMA queues
        nc.vector.tensor_copy(out=qTD[0][0:64, :], in_=qTA[0:64, :])
        nc.vector.tensor_copy(out=qTD[0][64:128, :], in_=qTA[0:64, :])
        nc.vector.tensor_copy(out=qTD[1][0:64, :], in_=qTA[64:128, :])
        nc.vector.tensor_copy(out=qTD[1][64:128, :], in_=qTA[64:128, :])
        nc.vector.tensor_copy(out=qTD[2][64:128, :], in_=qTD[2][0:64, :])
        nc.vector.tensor_copy(out=kTD[0][0:64, :], in_=kTA[0:64, :])
        nc.vector.tensor_copy(out=kTD[0][64:128, :], in_=kTA[0:64, :])
        nc.vector.tensor_copy(out=kTD[1][0:64, :], in_=kTA[64:128, :])
        nc.vector.tensor_copy(out=kTD[1][64:128, :], in_=kTA[64:128, :])
        nc.vector.tensor_copy(out=kTD[2][64:128, :], in_=kTD[2][0:64, :])

    # ---- phases 2+3: v projection + score stream share the PSUM pools ----
    def vh_ap(kj, h):
        base = (kj * HPC + h) * 65
        return v_sb[:, base:base + 65]

    # normalize phase 1: denom row -> sbuf, fast reciprocal, then a
    # gpsimd partition_broadcast (all-SBUF, so legal on Pool) replaces the
    # old DRAM DMA roundtrip.  The fused multiply (phase 2) is DEFERRED
    # two groups so its wait never head-of-line-blocks the DVE queue.
    def norm_start(yt, h, qq):
        den = small.tile([1, QB], F32, tag="den")
        nc.vector.tensor_copy(out=den, in_=yt[64:65, :])
        rec = small.tile([1, QB], F32, tag="rec")
        nc.vector.reciprocal_approx_fast(rec, den)
        bc = small.tile([64, QB], F32, tag="bc_sb")
        nc.gpsimd.partition_broadcast(bc, rec)
        return (yt, bc, h, qq)

    def norm_finish(state):
        yt, bc, h, qq = state
        q0 = qq * QB
        ydst = yTA[0:64, :] if h == 0 else (
            yTA[64:128, :] if h == 1 else yTB[0:64, :])
        nc.vector.scalar_tensor_tensor(
            out=ydst[:, q0:q0 + QB], in0=yt[0:64, :], scalar=1.0, in1=bc,
            op0=OP.mult, op1=OP.mult,
        )

    pj_state = {}

    def proj_half(ps_st, qt, nb):
        # half an output row-tile per call (one ~0.65us PE detour per
        # group keeps ST supply within the ACT engine's slack)
        if nb == 0:
            stt = ps_st.tile([128, 3 * QB], F32, tag="st", name=f"pj{qt}")
            ob = ostage.tile([128, C], MMDT, tag="ob", name=f"ob{qt}")
            pj_state[qt] = (stt, ob)
        stt, ob = pj_state[qt]
        po = stt[:, nb * QB: nb * QB + 384]
        nc.tensor.matmul(po, yTA[:, qt * 128:(qt + 1) * 128],
                         wpA[:, nb * 384:(nb + 1) * 384],
                         start=True, stop=False)
        nc.tensor.matmul(po, yTB[0:64, qt * 128:(qt + 1) * 128],
                         wpB[0:64, nb * 384:(nb + 1) * 384],
                         start=False, stop=True)
        nc.vector.tensor_copy(out=ob[:, nb * 384:(nb + 1) * 384], in_=po)
        if nb == 1:
            nc.sync.dma_start(out=out[qt * 128:(qt + 1) * 128, :], in_=ob)
            del pj_state[qt]

    stream = [(qq, h, kj)
              for qq in range(4) for h in range(HPC) for kj in range(NTT)]
    NG = len(stream) // 3  # 64 groups of 3 score tiles

    # proj for block qq interleaved into block qq+1's stream, one nb-half
    # per group; slot allocations stay 2 groups apart (parity-preserving);
    # keyed by CONSUMED group
    proj_at = {}
    for qq in range(3):
        for j, goff in enumerate((8, 10, 12, 14)):
            qt = qq * 4 + j
            proj_at.setdefault((qq + 1) * 16 + goff, []).append((qt, 0))
            proj_at.setdefault((qq + 1) * 16 + goff + 1, []).append((qt, 1))

    def emit_st_group(ps_st, g):
        entries = [stream[3 * g + j] for j in range(3)]
        st = ps_st.tile([128, 3 * QB], F32, tag="st", name=f"st{g}")
        for j, (qq, h, kj) in enumerate(entries):
            lo = 0 if kj % 2 == 0 else 64
            nc.tensor.matmul(
                st[:, j * QB:(j + 1) * QB],
                kTD[h][lo:lo + 64, kj * 128:(kj + 1) * 128],
                qTD[h][lo:lo + 64, qq * QB:(qq + 1) * QB],
                start=True, stop=True,
            )
        et = et_pool.tile([128, 3 * QB], MMDT, tag="et", name=f"et{g}")
        nc.scalar.activation(et, st, AF.Exp, bias=shift_col[:, :])
        return (et, entries)

    with tc.tile_pool(name="ps_st", bufs=2, space="PSUM") as ps_st, \
         tc.tile_pool(name="ps_yt", bufs=2, space="PSUM") as ps_yt:
        ring = []
        # prefetch the first LAG score groups: their exp runs under v-proj
        # (2 groups fill the st slots; 2 more slot in after a few v tiles
        # so the PE never queues behind an ACT wait)
        def emit_v(nt):
            ps = ps_yt.tile([128, MYC], F32, tag="yt", name=f"ps_v{nt}")
            for kc in range(KC):
                nc.tensor.matmul(
                    ps,
                    xT_sb[:, kc * N + nt * 128: kc * N + nt * 128 + 128],
                    wv_sb[:, kc * MYC:(kc + 1) * MYC],
                    start=(kc == 0), stop=False,
                )
            nc.tensor.matmul(ps, ones[0:1, 0:128], bv_row,
                             start=False, stop=True)
            for h in range(HPC):
                base = (nt * HPC + h) * 65
                nc.vector.tensor_copy(out=v_sb[:, base:base + 64],
                                      in_=ps[:, h * 64:(h + 1) * 64])
                nc.vector.tensor_copy(out=v_sb[:, base + 64:base + 65],
                                      in_=ones_f32)

        ring.append(emit_st_group(ps_st, 0))
        ring.append(emit_st_group(ps_st, 1))
        for nt in range(4):
            emit_v(nt)
        ring.append(emit_st_group(ps_st, 2))
        ring.append(emit_st_group(ps_st, 3))
        for nt in range(4, NTT):
            emit_v(nt)

        # ---- phase 3 main loop ----
        yt_cur = {}
        pending = []   # (due consumed-group, norm state)
        pjAB = []
        for gi in range(LAG, NG + LAG):
            if gi < NG:
                ring.append(emit_st_group(ps_st, gi))
            cg = gi - LAG
            pet, pentries = ring.pop(0)
            for j, (qq, h, kj) in enumerate(pentries):
                if kj == 0:
                    yt_cur[(qq, h)] = ps_yt.tile([65, QB], F32, tag="yt",
                                                 name=f"yt{qq}_{h}")
                nc.tensor.matmul(yt_cur[(qq, h)], vh_ap(kj, h),
                                 pet[:, j * QB:(j + 1) * QB],
                                 start=(kj == 0), stop=(kj == NTT - 1))
                if kj == NTT - 1:
                    pending.append(
                        (cg + 2, norm_start(yt_cur.pop((qq, h)), h, qq)))
            while pending and pending[0][0] <= cg:
                norm_finish(pending.pop(0)[1])
            for qt, nb in proj_at.get(cg + 1, []):
                proj_half(ps_st, qt, nb)
        # final block's h0+h1 proj contribution emitted after every other
        # st-pool user (the partA tiles hold both slots until partB):
        # 8 bank-aligned po regions (3+3 in the st slots, 2 from yt pool)
        stA = ps_st.tile([128, 3 * QB], F32, tag="st", name="pjtA")
        stB = ps_st.tile([128, 3 * QB], F32, tag="st", name="pjtB")
        poY = [ps_yt.tile([128, 384], F32, tag="yt", name=f"poY{_i}")
               for _i in range(2)]
        for p in range(8):
            qt, nb = 12 + p // 2, p % 2
            if p < 3:
                po = stA[:, p * QB: p * QB + 384]
            elif p < 6:
                po = stB[:, (p - 3) * QB: (p - 3) * QB + 384]
            else:
                po = poY[p - 6][:, 0:384]
            pjAB.append(po)
            nc.tensor.matmul(po, yTA[:, qt * 128:(qt + 1) * 128],
                             wpA[:, nb * 384:(nb + 1) * 384],
                             start=True, stop=False)
        while pending:
            norm_finish(pending.pop(0)[1])
        for qx in range(4):
            qt = 12 + qx
            ob = ostage.tile([128, C], MMDT, tag="ob", name=f"ob{qt}")
            for nb in range(2):
                po = pjAB[qx * 2 + nb]
                nc.tensor.matmul(po, yTB[0:64, qt * 128:(qt + 1) * 128],
                                 wpB[0:64, nb * 384:(nb + 1) * 384],
                                 start=False, stop=True)
                nc.vector.tensor_copy(out=ob[:, nb * 384:(nb + 1) * 384],
                                      in_=po)
            nc.sync.dma_start(out=out[qt * 128:(qt + 1) * 128, :], in_=ob)


def _build_program():
    nc = bacc.Bacc("TRN2", target_bir_lowering=False, debug=False,
                   num_devices=NCORES)
    aps = {
        "xT": nc.dram_tensor("xT", [C, N], MMDT, kind="ExternalInput").ap(),
        "wq": nc.dram_tensor("wq", [C, 128], MMDT, kind="ExternalInput").ap(),
        "wk": nc.dram_tensor("wk", [C, 128], MMDT, kind="ExternalInput").ap(),
        "wqk2": nc.dram_tensor("wqk2", [C, 128], MMDT,
                               kind="ExternalInput").ap(),
        "wv": nc.dram_tensor("wv", [C, MYC], MMDT, kind="ExternalInput").ap(),
        "wp": nc.dram_tensor("wp", [MYC, C], MMDT, kind="ExternalInput").ap(),
        "bq": nc.dram_tensor("bq", [MYC, 1], F32, kind="ExternalInput").ap(),
        "bk": nc.dram_tensor("bk", [MYC, 1], F32, kind="ExternalInput").ap(),
        "bv": nc.dram_tensor("bv", [1, MYC], MMDT, kind="ExternalInput").ap(),
        "out": nc.dram_tensor("out", [N, C], MMDT, kind="ExternalOutput").ap(),
    }
    with tile.TileContext(nc) as tc:
        import contextlib
        with contextlib.ExitStack() as ctx:
            pools = {
                "persist": ctx.enter_context(tc.tile_pool(name="persist", bufs=1)),
                "et": ctx.enter_context(tc.tile_pool(name="et", bufs=LAG + 1)),
                "small": ctx.enter_context(tc.tile_pool(name="small", bufs=3)),
                "ostage": ctx.enter_context(tc.tile_pool(name="ostage", bufs=2)),
            }
            _emit(nc, tc, pools, aps)
    nc.compile()
    return nc


_PROGRAM_CACHE = {}


def _get_program():
    if "nc" not in _PROGRAM_CACHE:
        _PROGRAM_CACHE["nc"] = _build_program()
    return _PROGRAM_CACHE["nc"]


def make_in_maps(x, Wq, bq, Wk, bk, Wv, bv, Wp, bp):
    scale = 1.0 / math.sqrt(DH)
    xTb = [np.ascontiguousarray(x[b].T) for b in range(B)]
    wire = mybir.dt.np(MMDT)
    in_maps = []
    for c in range(NCORES):
        b, hg = c // CPG, c % CPG
        cols = slice(hg * MYC, (hg + 1) * MYC)
        wqc = Wq[:, cols] * np.float32(scale)
        wkc = Wk[:, cols]
        in_maps.append({
            "xT": xTb[b].astype(wire),
            "wq": np.ascontiguousarray(wqc[:, 0:128]).astype(wire),
            "wk": np.ascontiguousarray(wkc[:, 0:128]).astype(wire),
            "wqk2": np.ascontiguousarray(
                np.concatenate([wqc[:, 128:192], wkc[:, 128:192]],
                               axis=1)).astype(wire),
            "wv": np.ascontiguousarray(Wv[:, cols]).astype(wire),
            "wp": np.ascontiguousarray(Wp[cols, :]).astype(wire),
            "bq": (bq[cols] * np.float32(scale)).reshape(MYC, 1).copy(),
            "bk": bk[cols].reshape(MYC, 1).copy(),
            "bv": bv[cols].reshape(1, MYC).astype(wire),
        })
    return in_maps


def assemble(results, bp):
    out = np.empty((B, N, C), np.float32)
    for b in range(B):
        acc = results[b * CPG]["out"].astype(np.float64)
        for c in range(b * CPG + 1, (b + 1) * CPG):
            acc = acc + results[c]["out"]
        out[b] = (acc + bp.astype(np.float64)).astype(np.float32)
    return out


def kernel(x, Wq, bq, Wk, bk, Wv, bv, Wp, bp, **extra_kwargs):
    x = np.asarray(x, np.float32)
    Wq = np.asarray(Wq, np.float32)
    Wk = np.asarray(Wk, np.float32)
    Wv = np.asarray(Wv, np.float32)
    Wp = np.asarray(Wp, np.float32)
    bq = np.asarray(bq, np.float32)
    bk = np.asarray(bk, np.float32)
    bv = np.asarray(bv, np.float32)
    bp = np.asarray(bp, np.float32)

    nc = _get_program()
    in_maps = make_in_maps(x, Wq, bq, Wk, bk, Wv, bv, Wp, bp)
    res = bass_utils.run_bass_kernel_spmd(nc, in_maps,
                                          core_ids=list(range(NCORES)))
    return assemble(res.results, bp)


# revision 24
# speedup vs baseline: 1.0712x; 1.0712x over previous
"""Multi-head attention (B=2, N=2048, C=768, H=12, DH=64) on 8 Trainium2 cores.

Sharding: data-parallel on batch (cores 0-3 -> b=0, cores 4-7 -> b=1),
tensor-parallel on heads within each group (3 heads/core: Wq/Wk/Wv column
slices, Wp row slices).  Each core emits its partial projection output
[N, C]; the host sums the 4 partials per batch and adds bp (cheaper than a
device collective at this size).

Per-core dataflow (feature-major, transpose-free, fp16 operands / fp32 psum):
  - host supplies xT = x[b].T  [C, N] in fp16; h2's q and k weight columns
    are host-packed into one [C,128] tensor so all qk-proj matmuls are M=128
  - qT,kT [64, N] per head = W.T @ xT; each head's 64 dims then duplicated
    onto both PE-row halves (SBUF->SBUF DMA) so score matmuls pair
    even/odd kj tiles on disjoint PE row halves (co-execute)
  - v [N, 192] token-major from xT as lhsT, with a ones column per head;
    v psum borrowed from the yt pool so the score stream's first groups
    (ST + exp) prefetch underneath the v projection (ET ring, LAG=4)
  - phase 3 is one continuous stream over (qq, head, kj): 192 score tiles
    STt [kj,qi] grouped 3 per [128,1536] psum tile; ONE exp ACT op per
    group; yT accumulation consumes ET four groups behind so transient PE
    detours (proj tiles, normalize) never starve the ACT engine
  - yT_aug[65, qi] = [v_h | 1].T @ ET accumulated over kj; row 64 = denom
  - normalize: denom -> sbuf copy, reciprocal_approx_fast, stride-0 DMA
    broadcast, fused multiply deferred two groups (no DVE head-of-line)
  - out[qi, C] partial = yT (stationary) @ Wp rows, interleaved into the
    next block's stream at slot-parity-preserving spacing; the last four
    row-tiles split h0+h1 (early) / h2 (after the final normalize)
"""

import math

import numpy as np

import concourse.bacc as bacc
import concourse.bass as bass
import concourse.mybir as mybir
import concourse.tile as tile
from concourse import bass_utils

B, N, C, H, DH = 2, 2048, 768, 12, 64
NCORES = 8
CPG = 4                  # cores per batch group
HPC = H // CPG           # heads per core = 3
MYC = HPC * DH           # per-core feature width = 192
KC = C // 128            # contraction chunks = 6
NTT = N // 128           # token tiles = 16
QB = 512                 # qi block (psum bank width, fp32)
LAG = 8                  # ET ring depth: yt consumption trails ACT by LAG
F32 = mybir.dt.float32
MMDT = mybir.dt.float16  # matmul operand dtype
AF = mybir.ActivationFunctionType
OP = mybir.AluOpType

EXP_SHIFT = -4.0         # exp(s + EXP_SHIFT); cancels between num and denom


def _bcast_parts(ap, nparts):
    """Partition-stride-0 broadcast view of a [1, F] AP (DMA source only)."""
    return bass.AP(tensor=ap.tensor, offset=ap.offset,
                   ap=[[0, nparts]] + [list(d) for d in ap.ap[1:]])


def _emit(nc, tc, pools, aps):
    xT, wq, wk, wqk2, wv, wp, bq, bk, bv, out = (
        aps["xT"], aps["wq"], aps["wk"], aps["wqk2"], aps["wv"], aps["wp"],
        aps["bq"], aps["bk"], aps["bv"], aps["out"],
    )
    persist = pools["persist"]
    et_pool = pools["et"]
    small = pools["small"]
    ostage = pools["ostage"]

    # ---- persistent SBUF tensors ----
    xT_sb = persist.tile([128, KC * N], MMDT, tag="xT_sb")
    wq_sb = persist.tile([128, KC * 128], MMDT, tag="wq_sb")
    wk_sb = persist.tile([128, KC * 128], MMDT, tag="wk_sb")
    wqk2_sb = persist.tile([128, KC * 128], MMDT, tag="wqk2_sb")
    wv_sb = persist.tile([128, KC * MYC], MMDT, tag="wv_sb")
    wpA = persist.tile([128, C], MMDT, tag="wpA")
    wpB = persist.tile([64, C], MMDT, tag="wpB")
    bqA = persist.tile([128, 1], F32, tag="bqA")
    bqB = persist.tile([64, 1], F32, tag="bqB")
    bkA = persist.tile([128, 1], F32, tag="bkA")
    bkB = persist.tile([64, 1], F32, tag="bkB")
    bv_row = persist.tile([1, MYC], MMDT, tag="bv_row")
    ones = persist.tile([1, 128], MMDT, tag="ones")
    shift_col = persist.tile([128, 1], F32, tag="shift_col")
    # compact projections (h0 on parts 0:64, h1 on 64:128; h2 separate)
    qTA = persist.tile([128, N], MMDT, tag="qTA")
    kTA = persist.tile([128, N], MMDT, tag="kTA")
    # partition-duplicated k/q halves for even/odd kj pair packing:
    # qTDx packs h0's dup (upper half) + h1's dup (lower half); h0-even
    # and h1-odd read qTA/kTA directly; h2 fully duplicated in qTD2
    qTDx = persist.tile([128, N], MMDT, tag="qTDx")
    kTDx = persist.tile([128, N], MMDT, tag="kTDx")
    qTD2 = persist.tile([128, N], MMDT, tag="qTD2")
    kTD2 = persist.tile([128, N], MMDT, tag="kTD2")
    v_sb = persist.tile([128, NTT * HPC * 65], MMDT, tag="v_sb")
    yTA = persist.tile([128, N], MMDT, tag="yTA")
    yTB = persist.tile([64, N], MMDT, tag="yTB")

    # ---- input DMAs: qk-proj operands first so phase 1 starts ASAP ----
    for kc in range(KC):
        nc.sync.dma_start(out=xT_sb[:, kc * N:(kc + 1) * N],
                          in_=xT[kc * 128:(kc + 1) * 128, :])
        nc.sync.dma_start(out=wq_sb[:, kc * 128:(kc + 1) * 128],
                          in_=wq[kc * 128:(kc + 1) * 128, :])
        nc.sync.dma_start(out=wk_sb[:, kc * 128:(kc + 1) * 128],
                          in_=wk[kc * 128:(kc + 1) * 128, :])
        nc.sync.dma_start(out=wqk2_sb[:, kc * 128:(kc + 1) * 128],
                          in_=wqk2[kc * 128:(kc + 1) * 128, :])
    nc.sync.dma_start(out=bqA, in_=bq[0:128, :])
    nc.sync.dma_start(out=bqB, in_=bq[128:MYC, :])
    nc.sync.dma_start(out=bkA, in_=bk[0:128, :])
    nc.sync.dma_start(out=bkB, in_=bk[128:MYC, :])
    for kc in range(KC):
        nc.sync.dma_start(out=wv_sb[:, kc * MYC:(kc + 1) * MYC],
                          in_=wv[kc * 128:(kc + 1) * 128, :])
    nc.sync.dma_start(out=bv_row, in_=bv)
    nc.sync.dma_start(out=wpA, in_=wp[0:128, :])
    nc.sync.dma_start(out=wpB, in_=wp[128:MYC, :])
    ones_f32 = persist.tile([128, 1], F32, tag="ones_f32")
    ones_row_f32 = persist.tile([1, 128], F32, tag="ones_row_f32")
    nc.vector.memset(ones_f32, 1.0)
    nc.vector.memset(ones_row_f32, 1.0)
    nc.vector.tensor_copy(out=ones, in_=ones_row_f32)
    nc.vector.memset(shift_col, EXP_SHIFT)

    # ---- phase 1: q/k/h2-combined projections (M=128 passes) ----
    with tc.tile_pool(name="ps_qk", bufs=2, space="PSUM") as ps_qk:
        pssQ = [ps_qk.tile([128, QB], F32, tag="ps_qkA", bufs=4,
                           name=f"ps_q{_i}") for _i in range(N // QB)]
        pssK = [ps_qk.tile([128, QB], F32, tag="ps_qkB", bufs=4,
                           name=f"ps_k{_i}") for _i in range(N // QB)]
        # q and k interleaved per kc chunk so matmul consumption stays
        # behind the xT DMA supply
        for kc in range(KC):
            for nt in range(N // QB):
                nc.tensor.matmul(
                    pssQ[nt], wq_sb[:, kc * 128:(kc + 1) * 128],
                    xT_sb[:, kc * N + nt * QB: kc * N + nt * QB + QB],
                    start=(kc == 0), stop=(kc == KC - 1))
            for nt in range(N // QB):
                nc.tensor.matmul(
                    pssK[nt], wk_sb[:, kc * 128:(kc + 1) * 128],
                    xT_sb[:, kc * N + nt * QB: kc * N + nt * QB + QB],
                    start=(kc == 0), stop=(kc == KC - 1))
        for nt in range(N // QB):
            nc.vector.tensor_scalar(
                out=qTA[:, nt * QB:(nt + 1) * QB], in0=pssQ[nt],
                scalar1=bqA, scalar2=None, op0=OP.add)
            nc.vector.tensor_scalar(
                out=kTA[:, nt * QB:(nt + 1) * QB], in0=pssK[nt],
                scalar1=bkA, scalar2=None, op0=OP.add)
        # combined h2 pass: psum rows 0:64 = q-h2, rows 64:128 = k-h2
        pss2 = [ps_qk.tile([128, QB], F32, tag="ps_qkA", bufs=4,
                           name=f"ps_2{_i}") for _i in range(N // QB)]
        for kc in range(KC):
            for nt in range(N // QB):
                nc.tensor.matmul(
                    pss2[nt], wqk2_sb[:, kc * 128:(kc + 1) * 128],
                    xT_sb[:, kc * N + nt * QB: kc * N + nt * QB + QB],
                    start=(kc == 0), stop=(kc == KC - 1))
        for nt in range(N // QB):
            nc.vector.tensor_scalar(
                out=qTD2[0:64, nt * QB:(nt + 1) * QB], in0=pss2[nt][0:64, :],
                scalar1=bqB, scalar2=None, op0=OP.add)
            nc.vector.tensor_scalar(
                out=kTD2[0:64, nt * QB:(nt + 1) * QB],
                in0=pss2[nt][64:128, :],
                scalar1=bkB, scalar2=None, op0=OP.add)
        # duplicated halves via DVE copies (4x SBUF fp16 mode; off the
        # loaded DMA queues): h0 -> upper of *TDx, h1 -> lower, h2 upper
        nc.vector.tensor_copy(out=qTDx[64:128, :], in_=qTA[0:64, :])
        nc.vector.tensor_copy(out=qTDx[0:64, :], in_=qTA[64:128, :])
        nc.vector.tensor_copy(out=kTDx[64:128, :], in_=kTA[0:64, :])
        nc.vector.tensor_copy(out=kTDx[0:64, :], in_=kTA[64:128, :])
        nc.vector.tensor_copy(out=qTD2[64:128, :], in_=qTD2[0:64, :])
        nc.vector.tensor_copy(out=kTD2[64:128, :], in_=kTD2[0:64, :])

    # ---- phases 2+3: v projection + score stream share the PSUM pools ----
    def vh_ap(kj, h):
        base = (kj * HPC + h) * 65
        return v_sb[:, base:base + 65]

    # normalize phase 1: denom row -> sbuf, fast reciprocal, then a
    # gpsimd partition_broadcast (all-SBUF, so legal on Pool) replaces the
    # old DRAM DMA roundtrip.  The fused multiply (phase 2) is DEFERRED
    # two groups so its wait never head-of-line-blocks the DVE queue.
    def norm_start(yt, h, qq):
        den = small.tile([1, QB], F32, tag="den")
        nc.vector.tensor_copy(out=den, in_=yt[64:65, :])
        rec = small.tile([1, QB], F32, tag="rec")
        nc.vector.reciprocal_approx_fast(rec, den)
        bc = small.tile([64, QB], F32, tag="bc_sb")
        nc.gpsimd.partition_broadcast(bc, rec)
        return (yt, bc, h, qq)

    def norm_finish(state):
        yt, bc, h, qq = state
        q0 = qq * QB
        ydst = yTA[0:64, :] if h == 0 else (
            yTA[64:128, :] if h == 1 else yTB[0:64, :])
        nc.vector.scalar_tensor_tensor(
            out=ydst[:, q0:q0 + QB], in0=yt[0:64, :], scalar=1.0, in1=bc,
            op0=OP.mult, op1=OP.mult,
        )

    pj_state = {}

    def proj_half(ps_st, qt, nb):
        # half an output row-tile per call (one ~0.65us PE detour per
        # group keeps ST supply within the ACT engine's slack)
        if nb == 0:
            stt = ps_st.tile([128, 3 * QB], F32, tag="st", name=f"pj{qt}")
            ob = ostage.tile([128, C], MMDT, tag="ob", name=f"ob{qt}")
            pj_state[qt] = (stt, ob)
        stt, ob = pj_state[qt]
        po = stt[:, nb * QB: nb * QB + 384]
        nc.tensor.matmul(po, yTA[:, qt * 128:(qt + 1) * 128],
                         wpA[:, nb * 384:(nb + 1) * 384],
                         start=True, stop=False)
        nc.tensor.matmul(po, yTB[0:64, qt * 128:(qt + 1) * 128],
                         wpB[0:64, nb * 384:(nb + 1) * 384],
                         start=False, stop=True)
        if nb == 1:
            # both casts after both matmul pairs: a cast between them would
            # serialize PE<->DVE on the shared stt tile (tile-level WAR)
            for b2 in range(2):
                nc.vector.tensor_copy(
                    out=ob[:, b2 * 384:(b2 + 1) * 384],
                    in_=stt[:, b2 * QB: b2 * QB + 384])
            nc.sync.dma_start(out=out[qt * 128:(qt + 1) * 128, :], in_=ob)
            del pj_state[qt]

    stream = [(qq, h, kj)
              for qq in range(4) for h in range(HPC) for kj in range(NTT)]
    NG = len(stream) // 3  # 64 groups of 3 score tiles

    # proj for block qq interleaved into block qq+1's stream, one nb-half
    # per group; slot allocations stay 2 groups apart (parity-preserving);
    # keyed by CONSUMED group
    proj_at = {}
    for qq in range(3):
        for j, goff in enumerate((8, 10, 12, 14)):
            qt = qq * 4 + j
            proj_at.setdefault((qq + 1) * 16 + goff, []).append((qt, 0))
            proj_at.setdefault((qq + 1) * 16 + goff + 1, []).append((qt, 1))

    def st_srcs(h, kj):
        if kj % 2 == 0:      # PE rows 0:64
            kt, qt_ = ((kTA, qTA), (kTDx, qTDx), (kTD2, qTD2))[h]
            lo = 0
        else:                # PE rows 64:128
            kt, qt_ = ((kTDx, qTDx), (kTA, qTA), (kTD2, qTD2))[h]
            lo = 64
        return kt, qt_, lo

    def emit_st_group(ps_st, g):
        entries = [stream[3 * g + j] for j in range(3)]
        st = ps_st.tile([128, 3 * QB], F32, tag="st", name=f"st{g}")
        for j, (qq, h, kj) in enumerate(entries):
            kt, qt_, lo = st_srcs(h, kj)
            nc.tensor.matmul(
                st[:, j * QB:(j + 1) * QB],
                kt[lo:lo + 64, kj * 128:(kj + 1) * 128],
                qt_[lo:lo + 64, qq * QB:(qq + 1) * QB],
                start=True, stop=True,
            )
        et = et_pool.tile([128, 3 * QB], MMDT, tag="et", name=f"et{g}")
        nc.scalar.activation(et, st, AF.Exp, bias=shift_col[:, :])
        return (et, entries)

    with tc.tile_pool(name="ps_st", bufs=2, space="PSUM") as ps_st, \
         tc.tile_pool(name="ps_yt", bufs=2, space="PSUM") as ps_yt:
        ring = []
        # prefetch the first LAG score groups: their exp runs under v-proj
        # (2 groups fill the st slots; 2 more slot in after a few v tiles
        # so the PE never queues behind an ACT wait)
        def emit_v(nt):
            ps = ps_yt.tile([128, MYC], F32, tag="yt", name=f"ps_v{nt}")
            for kc in range(KC):
                nc.tensor.matmul(
                    ps,
                    xT_sb[:, kc * N + nt * 128: kc * N + nt * 128 + 128],
                    wv_sb[:, kc * MYC:(kc + 1) * MYC],
                    start=(kc == 0), stop=False,
                )
            nc.tensor.matmul(ps, ones[0:1, 0:128], bv_row,
                             start=False, stop=True)
            for h in range(HPC):
                base = (nt * HPC + h) * 65
                nc.vector.tensor_copy(out=v_sb[:, base:base + 64],
                                      in_=ps[:, h * 64:(h + 1) * 64])
                nc.vector.tensor_copy(out=v_sb[:, base + 64:base + 65],
                                      in_=ones_f32)

        ring.append(emit_st_group(ps_st, 0))
        ring.append(emit_st_group(ps_st, 1))
        for k in range(2, LAG):
            s = NTT * (k - 2) // (LAG - 2)
            e = NTT * (k - 1) // (LAG - 2)
            for nt in range(s, e):
                emit_v(nt)
            ring.append(emit_st_group(ps_st, k))

        # ---- phase 3 main loop ----
        yt_cur = {}
        pending = []   # (due consumed-group, norm state)
        pjAB = []
        for gi in range(LAG, NG + LAG):
            if gi < NG:
                ring.append(emit_st_group(ps_st, gi))
            cg = gi - LAG
            pet, pentries = ring.pop(0)
            for j, (qq, h, kj) in enumerate(pentries):
                if kj == 0:
                    yt_cur[(qq, h)] = ps_yt.tile([65, QB], F32, tag="yt",
                                                 name=f"yt{qq}_{h}")
                nc.tensor.matmul(yt_cur[(qq, h)], vh_ap(kj, h),
                                 pet[:, j * QB:(j + 1) * QB],
                                 start=(kj == 0), stop=(kj == NTT - 1))
                if kj == NTT - 1:
                    pending.append(
                        (cg + 2, norm_start(yt_cur.pop((qq, h)), h, qq)))
            while pending and pending[0][0] <= cg:
                norm_finish(pending.pop(0)[1])
            for qt, nb in proj_at.get(cg + 1, []):
                proj_half(ps_st, qt, nb)
        # final block's h0+h1 proj contribution emitted after every other
        # st-pool user (the partA tiles hold both slots until partB):
        # 8 bank-aligned po regions (3+3 in the st slots, 2 from yt pool)
        stA = ps_st.tile([128, 3 * QB], F32, tag="st", name="pjtA")
        stB = ps_st.tile([128, 3 * QB], F32, tag="st", name="pjtB")
        poY = [ps_yt.tile([128, 384], F32, tag="yt", name=f"poY{_i}")
               for _i in range(2)]
        for p in range(8):
            qt, nb = 12 + p // 2, p % 2
            if p < 3:
                po = stA[:, p * QB: p * QB + 384]
            elif p < 6:
                po = stB[:, (p - 3) * QB: (p - 3) * QB + 384]
            else:
                po = poY[p - 6][:, 0:384]
            pjAB.append(po)
            nc.tensor.matmul(po, yTA[:, qt * 128:(qt + 1) * 128],
                             wpA[:, nb * 384:(nb + 1) * 384],
                             start=True, stop=False)
        while pending:
            norm_finish(pending.pop(0)[1])
        # all 8 partB matmuls first, then the casts (a cast between two
        # matmuls sharing a psum tile serializes PE<->DVE)
        for p in range(8):
            qt, nb = 12 + p // 2, p % 2
            nc.tensor.matmul(pjAB[p], yTB[0:64, qt * 128:(qt + 1) * 128],
                             wpB[0:64, nb * 384:(nb + 1) * 384],
                             start=False, stop=True)
        for qx in range(4):
            qt = 12 + qx
            ob = ostage.tile([128, C], MMDT, tag="ob", name=f"ob{qt}")
            for nb in range(2):
                nc.vector.tensor_copy(out=ob[:, nb * 384:(nb + 1) * 384],
                                      in_=pjAB[qx * 2 + nb])
            nc.sync.dma_start(out=out[qt * 128:(qt + 1) * 128, :], in_=ob)


def _build_program():
    nc = bacc.Bacc("TRN2", target_bir_lowering=False, debug=False,
                   num_devices=NCORES)
    aps = {
        "xT": nc.dram_tensor("xT", [C, N], MMDT, kind="ExternalInput").ap(),
        "wq": nc.dram_tensor("wq", [C, 128], MMDT, kind="ExternalInput").ap(),
        "wk": nc.dram_tensor("wk", [C, 128], MMDT, kind="ExternalInput").ap(),
        "wqk2": nc.dram_tensor("wqk2", [C, 128], MMDT,
                               kind="ExternalInput").ap(),
        "wv": nc.dram_tensor("wv", [C, MYC], MMDT, kind="ExternalInput").ap(),
        "wp": nc.dram_tensor("wp", [MYC, C], MMDT, kind="ExternalInput").ap(),
        "bq": nc.dram_tensor("bq", [MYC, 1], F32, kind="ExternalInput").ap(),
        "bk": nc.dram_tensor("bk", [MYC, 1], F32, kind="ExternalInput").ap(),
        "bv": nc.dram_tensor("bv", [1, MYC], MMDT, kind="ExternalInput").ap(),
        "out": nc.dram_tensor("out", [N, C], MMDT, kind="ExternalOutput").ap(),
    }
    with tile.TileContext(nc) as tc:
        import contextlib
        with contextlib.ExitStack() as ctx:
            pools = {
                "persist": ctx.enter_context(tc.tile_pool(name="persist", bufs=1)),
                "et": ctx.enter_context(tc.tile_pool(name="et", bufs=LAG + 1)),
                "small": ctx.enter_context(tc.tile_pool(name="small", bufs=3)),
                "ostage": ctx.enter_context(tc.tile_pool(name="ostage", bufs=2)),
            }
            _emit(nc, tc, pools, aps)
    nc.compile()
    return nc


_PROGRAM_CACHE = {}


def _get_program():
    if "nc" not in _PROGRAM_CACHE:
        _PROGRAM_CACHE["nc"] = _build_program()
    return _PROGRAM_CACHE["nc"]


def make_in_maps(x, Wq, bq, Wk, bk, Wv, bv, Wp, bp):
    scale = 1.0 / math.sqrt(DH)
    xTb = [np.ascontiguousarray(x[b].T) for b in range(B)]
    wire = mybir.dt.np(MMDT)
    in_maps = []
    for c in range(NCORES):
        b, hg = c // CPG, c % CPG
        cols = slice(hg * MYC, (hg + 1) * MYC)
        wqc = Wq[:, cols] * np.float32(scale)
        wkc = Wk[:, cols]
        in_maps.append({
            "xT": xTb[b].astype(wire),
            "wq": np.ascontiguousarray(wqc[:, 0:128]).astype(wire),
            "wk": np.ascontiguousarray(wkc[:, 0:128]).astype(wire),
            "wqk2": np.ascontiguousarray(
                np.concatenate([wqc[:, 128:192], wkc[:, 128:192]],
                               axis=1)).astype(wire),
            "wv": np.ascontiguousarray(Wv[:, cols]).astype(wire),
            "wp": np.ascontiguousarray(Wp[cols, :]).astype(wire),
            "bq": (bq[cols] * np.float32(scale)).reshape(MYC, 1).copy(),
            "bk": bk[cols].reshape(MYC, 1).copy(),
            "bv": bv[cols].reshape(1, MYC).astype(wire),
        })
    return in_maps


def assemble(results, bp):
    out = np.empty((B, N, C), np.float32)
    for b in range(B):
        acc = results[b * CPG]["out"].astype(np.float64)
        for c in range(b * CPG + 1, (b + 1) * CPG):
            acc = acc + results[c]["out"]
        out[b] = (acc + bp.astype(np.float64)).astype(np.float32)
    return out


def kernel(x, Wq, bq, Wk, bk, Wv, bv, Wp, bp, **extra_kwargs):
    x = np.asarray(x, np.float32)
    Wq = np.asarray(Wq, np.float32)
    Wk = np.asarray(Wk, np.float32)
    Wv = np.asarray(Wv, np.float32)
    Wp = np.asarray(Wp, np.float32)
    bq = np.asarray(bq, np.float32)
    bk = np.asarray(bk, np.float32)
    bv = np.asarray(bv, np.float32)
    bp = np.asarray(bp, np.float32)

    nc = _get_program()
    in_maps = make_in_maps(x, Wq, bq, Wk, bk, Wv, bv, Wp, bp)
    res = bass_utils.run_bass_kernel_spmd(nc, in_maps,
                                          core_ids=list(range(NCORES)))
    return assemble(res.results, bp)
